# revision 1
# baseline (speedup 1.0000x reference)
"""Trainium2 Bass kernel for the bidirectional RNN language model.

Model (see problem reference): for a [L=128, B=32] int token grid,
  - forward + backward tanh-RNN (HID=20) over EMB=80 embeddings (VOCAB=32000)
  - per position: logits = [h_fwd[i], h_bwd[i+1]] @ h2o   -> [*, 32000]
  - output log_softmax(logits)  ->  [128, 32, 32000] f32  (512 MB)

Strategy: data-parallel over batch across 8 NeuronCores (4 batch columns
per core), no collectives. Each core:
  - gathers its embeddings via indirect DMA (fwd + reversed streams) and
    PE-transposes them into [80, pos*batch] column layout,
  - runs fwd+bwd recurrences COMBINED: step tau computes
    [hiddenf[tau+1] | hiddenb[127-tau]] with one PE matmul (K=112,
    zero-padded for 32-aligned partition starts) + one ACT tanh whose
    output lands contiguously as step tau+1's matmul operand,
  - projects onto the vocab twice in float32r (full-rate fp32 matmul):
    pass 1 exp with fused per-row accum_out for the log-softmax
    normalizer (no max-subtraction needed: |logits| <= ~9, exp is safe
    in fp32, and logits are never materialized); pass 2 recomputes each
    chunk and subtracts ln(sum) on DVE straight into staging,
  - streams the [512, 32000] result to HBM in 2 MB chunks.
The projection is pipelined over four middle-out "position tiles" so the
first tile's pass-1 is woven between recurrence steps (ACT is in-order),
pass-2/DMA of tile k overlaps pass-1 of tile k+1 (PE order pinned via
add_dep_helper so the scheduler can't starve the DVE/DMA stream), and the
last tile's subtracts are split DVE/ACT. Cost-model exec: ~321 us/core,
memory-bound on the 512 MB output write from ~t=140 us on.
"""

import numpy as np

import concourse.bacc as bacc
import concourse.tile as tile
from concourse import bass, mybir
from concourse.bass_utils import run_bass_kernel_spmd
from concourse.masks import make_identity
from concourse.tile_rust import add_dep_helper

L = 128
B = 32
V = 32000
EMB = 80
HID = 20
KDIM = EMB + HID          # 100
# Device-side contraction layout: hidden rows at partitions 0:20, zero pad
# 20:32 (compute-engine APs must start 32-aligned), embeddings at 32:112.
EOFF = 32
KP = EOFF + EMB           # 112
H2 = 2 * HID              # 40
NCORES = 8
BL = B // NCORES          # 4 batch columns per core
R = L * BL                # 512 output rows per core
NT = 4                    # row-tiles of 128 rows (32 positions) per core

CH = 1024                 # vocab chunk per PSUM tile (2 banks)
NFULL = V // CH           # 31 full chunks
REM = V - NFULL * CH      # 256
NVC = NFULL + 1           # 32 chunks
GROUP = 4                 # chunks per output staging DMA (16 KB/partition)

F32 = mybir.dt.float32

_CACHE = {}

# Optional extra kwargs for run_bass_kernel_spmd (used by test harness for
# tracing); harmless defaults for grading.
RUN_KWARGS = {}
LAST_RESULTS = None


def _build():
    nc = bacc.Bacc("TRN2", debug=False, num_devices=NCORES)

    F32R = mybir.dt.float32r
    # idx rows 0..511: tokens in (position, batch) row-major order;
    # rows 512..1023: same with positions reversed (backward-chain gather).
    idx_d = nc.dram_tensor("idx", [2 * R, 1], mybir.dt.int32, kind="ExternalInput")
    we_d = nc.dram_tensor("we", [V, EMB], F32, kind="ExternalInput")
    i2h_d = nc.dram_tensor("i2h", [KP, HID], F32, kind="ExternalInput")
    # float32r: PE streams fp32 at full rate (1 cycle/row vs 4 for plain
    # fp32) with tf32-like operand truncation — ~2e-4 relative effect on
    # logits, far inside tolerance. Same bytes as fp32 host-side.
    h2o_d = nc.dram_tensor("h2o", [H2, V], F32R, kind="ExternalInput")
    biasc_d = nc.dram_tensor("biasc", [HID, 1], F32, kind="ExternalInput")
    h0r_d = nc.dram_tensor("h0r", [HID, 2 * BL], F32, kind="ExternalInput")
    out_d = nc.dram_tensor("out", [R, V], F32, kind="ExternalOutput")

    with tile.TileContext(nc) as tc:
        with (
            tc.tile_pool(name="const", bufs=1) as const,
            tc.tile_pool(name="hbuf", bufs=1) as hbuf,
            tc.tile_pool(name="gat", bufs=2) as gat,
            tc.tile_pool(name="stat", bufs=1) as stat,
            tc.tile_pool(name="stage", bufs=3) as stage,
        ):
            ident = const.tile([128, 128], F32)
            make_identity(nc, ident[:])
            # One strided DMA loads all eight gather index columns (4 fwd +
            # 4 bwd): idx8[p, k] = idx[128k + p].
            idx8 = const.tile([128, 2 * NT], mybir.dt.int32)
            nc.sync.dma_start(
                out=idx8[:],
                in_=bass.AP(tensor=idx_d, offset=0, ap=[[1, 128], [128, 2 * NT]]),
            )
            i2h_sb = const.tile([KP, HID], F32)
            nc.sync.dma_start(out=i2h_sb[:], in_=i2h_d[:, :])
            biasc = const.tile([HID, 1], F32)
            nc.sync.dma_start(out=biasc[:], in_=biasc_d[:, :])
            # h2o is loaded via gpsimd (SWDGE) so the sync queue stays free
            # for the output stream; emitted after the gathers below.
            h2o_sb = const.tile([H2, V], F32R)

            # Combined recurrence operand buffers: step tau's block (8 cols)
            # at tile tau//32, local cols 8*(tau%32): [fwd tau | bwd 127-tau].
            # Rows 0:20 = hidden inputs ([hiddenf[tau] | hiddenb[128-tau]]),
            # rows 20:32 zero pad, rows 32:112 = embedding^T
            # ([tok[tau] | tok[127-tau]]). One matmul + one tanh per step;
            # the tanh output [hiddenf[tau+1] | hiddenb[127-tau]] is exactly
            # block tau+1's hidden-input rows — contiguous.
            rhsC = [
                hbuf.tile([KP, 256], F32, name=f"rhsC{k}", tag=f"rhsC{k}")
                for k in range(NT)
            ]
            for k in range(NT):
                nc.vector.memset(rhsC[k][:, :], 0.0)
            nc.sync.dma_start(out=rhsC[0][0:HID, 0:8], in_=h0r_d[:, :])

            with tc.tile_pool(name="tps", bufs=2, space="PSUM") as tps:
                # --- embedding gathers (fwd + bwd streams) + transpose ---
                for k in range(NT):
                    for half, icol in ((0, k), (1, NT + k)):
                        embG = gat.tile([128, EMB], F32, tag="embG")
                        nc.gpsimd.indirect_dma_start(
                            out=embG[:],
                            out_offset=None,
                            in_=we_d[:, :],
                            in_offset=bass.IndirectOffsetOnAxis(
                                ap=idx8[:, icol : icol + 1], axis=0
                            ),
                        )
                        psT = tps.tile([EMB, 128], F32, tag="tp")
                        nc.tensor.transpose(
                            out=psT[:], in_=embG[:], identity=ident[:]
                        )
                        embT = gat.tile([EMB, 128], F32, tag="embT")
                        nc.vector.tensor_copy(out=embT[:], in_=psT[:])
                        dst = rhsC[k][EOFF:, :].rearrange(
                            "p (b g) -> p b g", g=8
                        )[:, :, 4 * half : 4 * half + 4]
                        nc.sync.dma_start(out=dst, in_=embT[:])
                # sync (HWDGE) handles f32r fine; gpsimd (SWDGE/Q7) crashes
                # the exec unit on float32r descriptors — keep these on sync,
                # emitted after the idx loads so the gathers aren't blocked.
                for q in range(4):
                    nc.sync.dma_start(
                        out=h2o_sb[:, q * (V // 4) : (q + 1) * (V // 4)],
                        in_=h2o_d[:, q * (V // 4) : (q + 1) * (V // 4)],
                    )

            # --- projection "position tiles": 128 output rows = 2 segments of
            # 16 positions each, chosen middle-out so the first tile's hidden
            # states (fwd AND bwd) complete earliest in the recurrence.
            # Segment start positions p0 per tile:
            PTS = [
                (48, 64),    # positions 48..79   (ready ~step 79)
                (16, 32),    # positions 16..47   (~step 110)
                (80, 96),    # positions 80..111  (~step 110)
                (0, 112),    # positions 0..15 + 112..127 (~step 126)
            ]
            hcatT = [
                hbuf.tile([H2, 128], F32R, name=f"hcatT{k}", tag=f"hcatT{k}")
                for k in range(NT)
            ]
            sparts = [
                stat.tile([128, NVC], F32, name=f"sparts{k}", tag=f"sparts{k}")
                for k in range(NT)
            ]
            logs = [
                stat.tile([128, 1], F32, name=f"logs{k}", tag=f"logs{k}")
                for k in range(NT)
            ]
            neg_logs = [
                stat.tile([128, 1], F32, name=f"nlogs{k}", tag=f"nlogs{k}")
                for k in range(NT)
            ]
            # P2 runs at 512-column granularity (single-bank PSUM tiles).
            p2_chunks = [(i * 512, 512) for i in range(V // 512)]
            if V % 512:
                p2_chunks.append(((V // 512) * 512, V % 512))

            with (
                tc.tile_pool(name="rps", bufs=2, space="PSUM") as rps,
                tc.tile_pool(name="p1ps", bufs=2, space="PSUM") as p1ps,
                tc.tile_pool(name="p2ps", bufs=2, space="PSUM") as p2ps,
            ):

                def emit_hcat(pt):
                    for s, p0 in enumerate(PTS[pt]):
                        d0 = 64 * s
                        # fwd: hiddenf[i] = block i's fwd hidden cols,
                        # ascending blocks.
                        kf, fc0 = p0 // 32, 8 * (p0 % 32)
                        tf = rhsC[kf]
                        src_f = bass.AP(
                            tensor=tf.tensor,
                            offset=tf.offset + fc0,
                            ap=[[tf.ap[0][0], HID], [8, 16], [1, 4]],
                        ).bitcast(F32R)
                        nc.sync.dma_start(
                            out=hcatT[pt][0:HID, d0 : d0 + 64], in_=src_f
                        )
                        # bwd: hiddenb[i+1] = block (127-i)'s bwd hidden cols,
                        # descending blocks (negative stride).
                        b_hi = 127 - p0
                        kb, bc0 = b_hi // 32, 8 * (b_hi % 32) + 4
                        tb = rhsC[kb]
                        src_b = bass.AP(
                            tensor=tb.tensor,
                            offset=tb.offset + bc0,
                            ap=[[tb.ap[0][0], HID], [-8, 16], [1, 4]],
                        ).bitcast(F32R)
                        nc.sync.dma_start(
                            out=hcatT[pt][HID:, d0 : d0 + 64], in_=src_b
                        )

                def emit_p1(pt, vc, after=None):
                    v0 = vc * CH
                    w = CH if vc < NFULL else REM
                    p1t = p1ps.tile([128, CH], F32, tag="p1", name="p1t")
                    for m in range(0, w, 512):
                        mw = min(512, w - m)
                        mm = nc.tensor.matmul(
                            out=p1t[:, m : m + mw],
                            lhsT=hcatT[pt][:],
                            rhs=h2o_sb[:, v0 + m : v0 + m + mw],
                            start=True,
                            stop=True,
                        )
                        if after is not None:
                            # Pin PE order: keep this pass-1 matmul behind the
                            # paired pass-2 matmul so the scheduler can't
                            # starve the DVE/DMA stream by hoisting P1 work.
                            add_dep_helper(
                                mm.ins, after.ins, sync=False,
                                reason="wave interleave order",
                            )
                            after = None
                    # exp in-place on the PSUM tile (output discarded; only the
                    # fused per-partition sum matters).
                    nc.scalar.activation(
                        out=p1t[:, :w],
                        in_=p1t[:, :w],
                        func=mybir.ActivationFunctionType.Exp,
                        accum_out=sparts[pt][:, vc : vc + 1],
                    )

                def emit_stats(pt):
                    # Sum the 32 per-chunk partials on ACT (Copy + accum_out),
                    # NOT on DVE: a DVE reduce here gets hoisted ahead of
                    # pending subtracts by the scheduler and head-of-line
                    # blocks DVE's in-order queue for the whole exp phase.
                    s_t = stat.tile([128, 1], F32, name=f"s{pt}", tag=f"s{pt}")
                    s_scr = stat.tile(
                        [128, NVC], F32, name=f"sscr{pt}", tag="sscr"
                    )
                    nc.scalar.activation(
                        out=s_scr[:],
                        in_=sparts[pt][:, :],
                        func=mybir.ActivationFunctionType.Copy,
                        accum_out=s_t[:],
                    )
                    nc.scalar.activation(
                        out=logs[pt][:],
                        in_=s_t[:],
                        func=mybir.ActivationFunctionType.Ln,
                    )
                    if pt == NT - 1:
                        nc.scalar.mul(out=neg_logs[pt][:], in_=logs[pt][:], mul=-1.0)

                state = {"stg": None, "goff": 0, "gw": 0}
                SGW = GROUP * CH  # staging width (4096)

                def emit_p2(pt, ci, act_share=False):
                    v0, w = p2_chunks[ci]
                    p2t = p2ps.tile([128, 512], F32, tag="p2", name="p2t")
                    mm = nc.tensor.matmul(
                        out=p2t[:, :w],
                        lhsT=hcatT[pt][:],
                        rhs=h2o_sb[:, v0 : v0 + w],
                        start=True,
                        stop=True,
                    )
                    off = v0 % SGW
                    if off == 0:
                        state["stg"] = stage.tile(
                            [128, SGW], F32, tag="stg", name="stg"
                        )
                        state["goff"] = v0
                    state["gw"] = off + w
                    if act_share:
                        # Final wave: ACT is otherwise idle — let it take the
                        # odd chunks (out = Identity(in) + (-ln s)).
                        nc.scalar.activation(
                            out=state["stg"][:, off : off + w],
                            in_=p2t[:, :w],
                            func=mybir.ActivationFunctionType.Identity,
                            bias=neg_logs[pt][:],
                        )
                    else:
                        nc.vector.tensor_scalar(
                            out=state["stg"][:, off : off + w],
                            in0=p2t[:, :w],
                            scalar1=logs[pt][:],
                            scalar2=None,
                            op0=mybir.AluOpType.subtract,
                        )
                    if off + w == SGW or ci == len(p2_chunks) - 1:
                        g0, gw = state["goff"], state["gw"]
                        r0a, r0b = 4 * PTS[pt][0], 4 * PTS[pt][1]
                        dst = bass.AP(
                            tensor=out_d,
                            offset=r0a * V + g0,
                            ap=[[(r0b - r0a) * V, 2], [V, 64], [1, gw]],
                        )
                        nc.sync.dma_start(out=dst, in_=state["stg"][:, :gw])
                    return mm

                # --- combined fwd+bwd recurrence: step tau computes
                # [hiddenf[tau+1] | hiddenb[127-tau]] with ONE matmul + ONE
                # tanh, written directly as block tau+1's hidden-input rows.
                # The first position-tile's pass-1 chunks are woven into the
                # emission stream: ACT is in-order, so exp work must be
                # emitted between tanh steps to overlap the recurrence tail.
                pt0_emitted = 0
                for step in range(L - 1):
                    k0, c0 = step // 32, 8 * (step % 32)
                    pc = rps.tile([HID, 2 * BL], F32, tag="rec")
                    nc.tensor.matmul(
                        out=pc[:],
                        lhsT=i2h_sb[:],
                        rhs=rhsC[k0][:, c0 : c0 + 8],
                        start=True,
                        stop=True,
                    )
                    t1 = step + 1
                    k1, c1 = t1 // 32, 8 * (t1 % 32)
                    nc.scalar.activation(
                        out=rhsC[k1][0:HID, c1 : c1 + 8],
                        in_=pc[:],
                        func=mybir.ActivationFunctionType.Tanh,
                        bias=biasc[:],
                    )
                    if step == 79:
                        emit_hcat(0)
                    if step >= 81 and pt0_emitted < 32:
                        emit_p1(0, pt0_emitted)
                        pt0_emitted += 1

                # --- remaining projection, pipelined over position tiles:
                # P1(pt+1) interleaves with P2(pt) so ACT (exp) and DVE
                # (subtract) run concurrently. PSUM: 2 rec + 4 p1 + 2 p2 = 8.
                for vc in range(pt0_emitted, NVC):
                    emit_p1(0, vc)
                for pt in range(NT):
                    emit_stats(pt)
                    # hcat(1) and hcat(2) are both ready before wave 0's
                    # output DMAs occupy the sync queue, so front-load them;
                    # hcat(3) goes at wave 1's start.
                    if pt == 0:
                        emit_hcat(1)
                        emit_hcat(2)
                    elif pt == 1:
                        emit_hcat(3)
                    last_wave = pt == NT - 1
                    for i in range(NVC):
                        last_mm = emit_p2(pt, 2 * i)
                        if 2 * i + 1 < len(p2_chunks):
                            last_mm = emit_p2(pt, 2 * i + 1, act_share=last_wave)
                        if pt + 1 < NT:
                            emit_p1(pt + 1, i, after=last_mm)

    nc.compile()
    return nc


def _get_nc():
    if "nc" not in _CACHE:
        _CACHE["nc"] = _build()
    return _CACHE["nc"]


def kernel(input, we, i2h, h2o, bias, h0):
    global LAST_RESULTS
    input = np.asarray(input)
    we = np.ascontiguousarray(np.asarray(we), dtype=np.float32)
    i2h = np.ascontiguousarray(np.asarray(i2h), dtype=np.float32)
    h2o = np.ascontiguousarray(np.asarray(h2o), dtype=np.float32)
    bias = np.asarray(bias, dtype=np.float32)
    h0 = np.asarray(h0, dtype=np.float32)

    biasc = np.ascontiguousarray(bias.reshape(1, HID).T)          # [20, 1]
    h0r = np.ascontiguousarray(
        np.repeat(h0.reshape(1, HID).T, 2 * BL, axis=1)           # [20, 8]
    )
    # Reorder i2h into the padded device contraction layout: hidden-state
    # weight rows first, zeros, then embedding weight rows.
    i2h_dev = np.zeros((KP, HID), dtype=np.float32)
    i2h_dev[0:HID] = i2h[EMB:]
    i2h_dev[EOFF:] = i2h[0:EMB]

    nc = _get_nc()
    in_maps = []
    for c in range(NCORES):
        tok = input[:, BL * c : BL * (c + 1)].astype(np.int32)    # [L, BL]
        idx = np.ascontiguousarray(
            np.concatenate([tok.reshape(R), tok[::-1].reshape(R)]).reshape(
                2 * R, 1
            )
        )
        in_maps.append(
            {
                "idx": idx,
                "we": we,
                "i2h": i2h_dev,
                "h2o": h2o,
                "biasc": biasc,
                "h0r": h0r,
            }
        )

    res = run_bass_kernel_spmd(
        nc, in_maps, core_ids=list(range(NCORES)), **RUN_KWARGS
    )
    LAST_RESULTS = res
    parts = [res.results[c]["out"].reshape(L, BL, V) for c in range(NCORES)]
    return np.concatenate(parts, axis=1)

